# revision 1
# baseline (speedup 1.0000x reference)
"""Trainium2 Bass kernel for nn_MoETransformerDecoderFFN_84026740178981.

Expert-parallel across 8 NeuronCores: core e computes expert e over the full
batch (gating replicated on every core in fp32; everything heavy in fp32r),
host sums the 8 per-core weighted outputs.

Self-contained: builds + compiles the Bass program on first call (cached at
module level), runs via PJRT on jax.devices()[:8].
"""
import sys

if '/opt/trn_rl_repo' not in sys.path:
    sys.path.insert(0, '/opt/trn_rl_repo')

import numpy as np

from contextlib import ExitStack

import concourse.bass as bass
import concourse.mybir as mybir
import concourse.tile as tile
from concourse import bacc

F32 = mybir.dt.float32
F32R = mybir.dt.float32r
AF = mybir.ActivationFunctionType
ALU = mybir.AluOpType


def _butterfly(nc, pool, val, nrows, op, nt):
    """In-place butterfly over partitions 0..nrows (power of 2) of val[32, nt].

    After this, rows 0..nrows all hold the op-reduction of the original rows.
    Rows nrows..32 of val must be initialized (memset) before calling.
    """
    stride = nrows // 2
    while stride >= 1:
        tmp = pool.tile([32, nt], val.dtype, tag="bfly_tmp")
        mask = [(i ^ stride) if i < nrows else i for i in range(32)]
        nc.vector.stream_shuffle(tmp, val[:, :], mask)
        nc.vector.tensor_tensor(val[0:nrows, :], val[0:nrows, :], tmp[0:nrows, :], op)
        stride //= 2


def _bcast(nc, out_tile, row_ap, parts):
    """Broadcast a [1, N] SBUF row across `parts` partitions via GPSIMD."""
    nc.gpsimd.partition_broadcast(out_tile, row_ap, channels=parts)


def build(B=4, S=1024, D=512, F=2048, E=8, NH=8, n_cores=8, NT=512,
          gelu_func=None, loop=1):
    HD = D // NH            # head dim (64)
    HPT = 128 // HD         # heads per 128-partition tile (2)
    T = B * S
    KC = D // 128           # feature chunks of D (4)
    FC = F // 128           # feature chunks of F (16)
    NT = min(NT, S)
    SB = S // NT            # token blocks per batch
    NKT = S // 128          # k-token tiles per batch (8)
    TB = T // NT            # total token blocks
    JT = D // 128           # output-feature tiles of D (4)
    eps = 1e-5
    if gelu_func is None:
        gelu_func = AF.Gelu

    nc = bacc.Bacc("TRN2", target_bir_lowering=False, debug=False,
                   num_devices=n_cores)

    # ---- DRAM I/O ----
    d_xT = nc.dram_tensor("xT", [D, T], F32R, kind="ExternalInput")
    d_gwkT = nc.dram_tensor("gwkT", [D, D], F32, kind="ExternalInput")
    d_gbk = nc.dram_tensor("gbk", [D], F32, kind="ExternalInput")
    d_eqT = nc.dram_tensor("eqT", [D, E], F32, kind="ExternalInput")  # pre-scaled D^-.5
    d_sel = nc.dram_tensor("sel", [E], F32, kind="ExternalInput")      # one-hot(expert)
    d_wqT = nc.dram_tensor("wqT", [D, D], F32R, kind="ExternalInput")  # pre-scaled HD^-.5
    d_wkT = nc.dram_tensor("wkT", [D, D], F32R, kind="ExternalInput")
    d_wvT = nc.dram_tensor("wvT", [D, D], F32R, kind="ExternalInput")
    d_bq = nc.dram_tensor("bq", [D], F32, kind="ExternalInput")        # pre-scaled
    d_bk = nc.dram_tensor("bk", [D], F32, kind="ExternalInput")
    d_bv = nc.dram_tensor("bv", [D], F32, kind="ExternalInput")
    d_woT = nc.dram_tensor("woT", [D, D], F32R, kind="ExternalInput")
    d_bo = nc.dram_tensor("bo", [D], F32, kind="ExternalInput")
    d_g1 = nc.dram_tensor("g1", [D], F32, kind="ExternalInput")
    d_be1 = nc.dram_tensor("be1", [D], F32, kind="ExternalInput")
    d_w1T = nc.dram_tensor("w1T", [D, F], F32R, kind="ExternalInput")
    d_bf1 = nc.dram_tensor("bf1", [F], F32, kind="ExternalInput")
    d_w2T = nc.dram_tensor("w2T", [F, D], F32R, kind="ExternalInput")
    d_bf2 = nc.dram_tensor("bf2", [D], F32, kind="ExternalInput")
    d_g2 = nc.dram_tensor("g2", [D], F32, kind="ExternalInput")
    d_be2 = nc.dram_tensor("be2", [D], F32, kind="ExternalInput")
    d_yT = nc.dram_tensor("yT", [D, T], F32, kind="ExternalOutput")
    d_ln1 = nc.dram_tensor("ln1d", [D, T], F32R)      # internal bounce
    d_g = nc.dram_tensor("gated", [T], F32)           # own gate weight per token
    d_g2d = d_g.rearrange("(o t) -> o t", o=1)        # [1, T] view

    def pcol(dram_1d, c):
        """[len] DRAM vector -> [128, c] partition-major view."""
        return dram_1d.rearrange("(c p) -> p c", p=128)

    with tile.TileContext(nc) as tc, ExitStack() as top:
        const = top.enter_context(tc.tile_pool(name="const", bufs=1))

        # ---- small whole-kernel constants ----
        ones_f32 = const.tile([128, 1], F32)
        nc.vector.memset(ones_f32, 1.0)
        ones_sb = const.tile([128, 1], F32R)
        nc.vector.tensor_copy(ones_sb, ones_f32)
        eps_sb = const.tile([1, 1], F32)
        nc.vector.memset(eps_sb, eps)
        sel_sb = const.tile([32, 1], F32)
        nc.vector.memset(sel_sb, 0.0)
        nc.sync.dma_start(out=sel_sb[0:E, 0:1],
                          in_=d_sel.rearrange("(e o) -> e o", o=1))
        g2_sb = const.tile([128, KC], F32)
        nc.sync.dma_start(out=g2_sb, in_=pcol(d_g2, KC))
        be2_sb = const.tile([128, KC], F32)
        nc.sync.dma_start(out=be2_sb, in_=pcol(d_be2, KC))
        bf1_sb = const.tile([128, FC], F32)
        nc.sync.dma_start(out=bf1_sb, in_=pcol(d_bf1, FC))
        bf2_sb = const.tile([128, KC], F32)
        nc.sync.dma_start(out=bf2_sb, in_=pcol(d_bf2, KC))

        for _loop in range(loop):
            # ================= phase 1 =================
            with ExitStack() as p1:
                pw = p1.enter_context(tc.tile_pool(name="pw", bufs=1))
                pool = p1.enter_context(tc.tile_pool(name="p1", bufs=1))
                pool2 = p1.enter_context(tc.tile_pool(name="p1b", bufs=2))
                pool3 = p1.enter_context(tc.tile_pool(name="p1c", bufs=3))
                psA = p1.enter_context(tc.tile_pool(name="psA", bufs=3, space="PSUM"))
                psS = p1.enter_context(tc.tile_pool(name="psS", bufs=2, space="PSUM"))
                psC = p1.enter_context(tc.tile_pool(name="psC", bufs=2, space="PSUM"))

                # ---- phase-1 weights (released before phase 2) ----
                gwk_sb = pw.tile([128, KC, D], F32)
                nc.sync.dma_start(out=gwk_sb,
                                  in_=d_gwkT.rearrange("(c p) j -> p c j", p=128))
                eq_sb = pw.tile([128, KC, E], F32)
                nc.sync.dma_start(out=eq_sb,
                                  in_=d_eqT.rearrange("(c p) e -> p c e", p=128))
                wq_sb = pw.tile([128, KC, D], F32R)
                nc.sync.dma_start(out=wq_sb,
                                  in_=d_wqT.rearrange("(c p) j -> p c j", p=128))
                wk_sb = pw.tile([128, KC, D], F32R)
                nc.sync.dma_start(out=wk_sb,
                                  in_=d_wkT.rearrange("(c p) j -> p c j", p=128))
                wv_sb = pw.tile([128, KC, D], F32R)
                nc.sync.dma_start(out=wv_sb,
                                  in_=d_wvT.rearrange("(c p) j -> p c j", p=128))
                wo_sb = pw.tile([128, KC, D], F32R)
                nc.sync.dma_start(out=wo_sb,
                                  in_=d_woT.rearrange("(c p) j -> p c j", p=128))
                gbk_sb = pw.tile([128, KC], F32)
                nc.sync.dma_start(out=gbk_sb, in_=pcol(d_gbk, KC))
                bq_sb = pw.tile([128, KC], F32)
                nc.sync.dma_start(out=bq_sb, in_=pcol(d_bq, KC))
                bk_sb = pw.tile([128, KC], F32)
                nc.sync.dma_start(out=bk_sb, in_=pcol(d_bk, KC))
                bo_sb = pw.tile([128, KC], F32)
                nc.sync.dma_start(out=bo_sb, in_=pcol(d_bo, KC))
                g1_sb = pw.tile([128, KC], F32)
                nc.sync.dma_start(out=g1_sb, in_=pcol(d_g1, KC))
                be1_sb = pw.tile([128, KC], F32)
                nc.sync.dma_start(out=be1_sb, in_=pcol(d_be1, KC))
                bv_bc = pw.tile([128, D], F32)
                nc.sync.dma_start(
                    out=bv_bc.rearrange("p (o d) -> p o d", o=1),
                    in_=d_bv.rearrange("(o d) -> o d", o=1).partition_broadcast(128))

                for b in range(B):
                    tok0 = b * S
                    xTb = pool.tile([128, KC, S], F32R, tag="xTb")
                    nc.sync.dma_start(
                        out=xTb,
                        in_=d_xT.rearrange("(c p) t -> p c t", p=128)[:, :, tok0:tok0 + S])


                    # ---- gating ----
                    for qb in range(SB):
                        ts = bass.ts(qb, NT)
                        xTbf = pool.tile([128, KC, NT], F32, tag="xTbf")
                        nc.gpsimd.dma_start(
                            out=xTbf,
                            in_=d_xT.rearrange("(c p) t -> p c t", p=128)[
                                :, :, tok0 + qb * NT:tok0 + (qb + 1) * NT])
                        keysT = pool.tile([128, KC, NT], F32, tag="keysT")
                        for jt in range(JT):
                            ps = psA.tile([128, NT], F32, tag="proj")
                            for kc in range(KC):
                                nc.tensor.matmul(
                                    ps, gwk_sb[:, kc, bass.ts(jt, 128)],
                                    xTbf[:, kc, :],
                                    start=(kc == 0), stop=(kc == KC - 1))
                            nc.vector.tensor_scalar_add(
                                keysT[:, jt, :], ps, gbk_sb[:, jt:jt + 1])
                        psg = psS.tile([E, NT], F32, tag="small")
                        for kc in range(KC):
                            nc.tensor.matmul(psg, eq_sb[:, kc, :], keysT[:, kc, :],
                                             start=(kc == 0), stop=(kc == KC - 1))
                        gsc = pool.tile([32, NT], F32, tag="gsc")
                        nc.vector.tensor_copy(gsc[0:E, :], psg)
                        # max1
                        m1 = pool.tile([32, NT], F32, tag="gm1")
                        nc.vector.memset(m1, 0.0)
                        nc.vector.tensor_copy(m1[0:E, :], gsc[0:E, :])
                        _butterfly(nc, pool, m1, E, ALU.max, NT)
                        # knock out the max, then max2
                        ge = pool.tile([32, NT], F32, tag="gge")
                        nc.vector.tensor_tensor(ge[0:E, :], gsc[0:E, :], m1[0:E, :],
                                                ALU.is_ge)
                        m2 = pool.tile([32, NT], F32, tag="gm2")
                        nc.vector.memset(m2, 0.0)
                        nc.vector.scalar_tensor_tensor(
                            m2[0:E, :], ge[0:E, :], -1e9, gsc[0:E, :],
                            ALU.mult, ALU.add)
                        _butterfly(nc, pool, m2, E, ALU.max, NT)
                        mask = pool.tile([32, NT], F32, tag="gmask")
                        nc.vector.tensor_tensor(mask[0:E, :], gsc[0:E, :], m2[0:E, :],
                                                ALU.is_ge)
                        # probs = exp(s)/Z
                        pe = pool.tile([32, NT], F32, tag="gpe")
                        nc.scalar.activation(pe[0:E, :], gsc[0:E, :], AF.Exp)
                        z = pool.tile([32, NT], F32, tag="gz")
                        nc.vector.memset(z, 0.0)
                        nc.vector.tensor_copy(z[0:E, :], pe[0:E, :])
                        _butterfly(nc, pool, z, E, ALU.add, NT)
                        zr = pool.tile([32, NT], F32, tag="gzr")
                        nc.vector.reciprocal(zr[0:E, :], z[0:E, :])
                        # gated = probs*mask ; renormalize ; select own row
                        gt = pool.tile([32, NT], F32, tag="ggt")
                        nc.vector.memset(gt, 0.0)
                        nc.vector.tensor_mul(gt[0:E, :], pe[0:E, :], zr[0:E, :])
                        nc.vector.tensor_mul(gt[0:E, :], gt[0:E, :], mask[0:E, :])
                        dn = pool.tile([32, NT], F32, tag="gdn")
                        nc.vector.memset(dn, 0.0)
                        nc.vector.tensor_copy(dn[0:E, :], gt[0:E, :])
                        _butterfly(nc, pool, dn, E, ALU.add, NT)
                        nc.vector.tensor_scalar_add(dn[0:E, :], dn[0:E, :], 1e-9)
                        dr = pool.tile([32, NT], F32, tag="gdr")
                        nc.vector.reciprocal(dr[0:E, :], dn[0:E, :])
                        nc.vector.tensor_mul(gt[0:E, :], gt[0:E, :], dr[0:E, :])
                        nc.vector.tensor_scalar_mul(gt[0:E, :], gt[0:E, :],
                                                    sel_sb[0:E, 0:1])
                        _butterfly(nc, pool, gt, E, ALU.add, NT)
                        nc.sync.dma_start(
                            out=d_g2d[0:1, tok0 + qb * NT:tok0 + (qb + 1) * NT],
                            in_=gt[0:1, :])

                    # ---- v projection (token-major, ones column per head) ----
                    v_t = pool.tile([128, NKT, NH, HD + 1], F32R, tag="v_t")
                    nc.vector.tensor_copy(
                        v_t[:, :, :, HD:HD + 1],
                        ones_f32.broadcast_to([128, NKT, NH, 1]))
                    for tt in range(NKT):
                        ps = psA.tile([128, D], F32, tag="proj")
                        for kc in range(KC):
                            nc.tensor.matmul(
                                ps, xTb[:, kc, bass.ts(tt, 128)], wv_sb[:, kc, :],
                                start=(kc == 0), stop=(kc == KC - 1))
                        nc.vector.tensor_add(
                            v_t[:, tt, :, 0:HD],
                            ps.rearrange("p (h d) -> p h d", h=NH),
                            bv_bc.rearrange("p (h d) -> p h d", h=NH))

                    # ---- ctx accumulator tile ----
                    ctxT = pool.tile([128, KC, S], F32R, tag="ctxT")

                    # ---- per head-pair: project q/k, then attention ----
                    for jt in range(JT):
                        qp = pool.tile([128, S], F32R, tag="qp")
                        kp = pool.tile([128, S], F32R, tag="kp")
                        for qb in range(SB):
                            ts = bass.ts(qb, NT)
                            psq = psA.tile([128, NT], F32, tag="proj")
                            for kc in range(KC):
                                nc.tensor.matmul(
                                    psq, wq_sb[:, kc, bass.ts(jt, 128)], xTb[:, kc, ts],
                                    start=(kc == 0), stop=(kc == KC - 1))
                            nc.vector.tensor_scalar_add(qp[:, ts], psq,
                                                        bq_sb[:, jt:jt + 1])
                            psk = psA.tile([128, NT], F32, tag="proj")
                            for kc in range(KC):
                                nc.tensor.matmul(
                                    psk, wk_sb[:, kc, bass.ts(jt, 128)], xTb[:, kc, ts],
                                    start=(kc == 0), stop=(kc == KC - 1))
                            nc.vector.tensor_scalar_add(kp[:, ts], psk,
                                                        bk_sb[:, jt:jt + 1])
                        for hh in range(HPT):
                            h = jt * HPT + hh
                            hp = bass.ds(hh * HD, HD)
                            for qb in range(SB):
                                ts = bass.ts(qb, NT)
                                psc = psC.tile([HD + 1, NT], F32, tag="ctx")
                                for ki in range(NKT):
                                    pss = psS.tile([128, NT], F32, tag="small")
                                    nc.tensor.matmul(
                                        pss, kp[hp, bass.ts(ki, 128)], qp[hp, ts],
                                        start=True, stop=True)
                                    pt = pool3.tile([128, NT], F32R, tag="pt")
                                    nc.scalar.activation(pt, pss, AF.Exp)
                                    nc.tensor.matmul(
                                        psc, v_t[:, ki, h, :], pt,
                                        start=(ki == 0), stop=(ki == NKT - 1))
                                rrow = pool3.tile([1, NT], F32, tag="rrow")
                                nc.vector.reciprocal(rrow, psc[HD:HD + 1, :])
                                rb = pool3.tile([HD, NT], F32, tag="rb")
                                _bcast(nc, rb, rrow, HD)
                                nc.vector.tensor_mul(
                                    ctxT[bass.ds(hh * HD, HD), jt, ts],
                                    psc[0:HD, :], rb)

                    # ---- output proj + residual + LN1 ----
                    for qb in range(SB):
                        ts = bass.ts(qb, NT)
                        r1 = pool.tile([128, KC, NT], F32R, tag="r1")
                        sq = pool.tile([128, KC, NT], F32R, tag="sq")
                        for jt in range(JT):
                            ps = psA.tile([128, NT], F32, tag="proj")
                            for kc in range(KC):
                                nc.tensor.matmul(
                                    ps, wo_sb[:, kc, bass.ts(jt, 128)],
                                    ctxT[:, kc, ts],
                                    start=(kc == 0), stop=(kc == KC - 1))
                            # r1 = (psum + bo) + x
                            nc.vector.scalar_tensor_tensor(
                                r1[:, jt, :], ps, bo_sb[:, jt:jt + 1],
                                xTb[:, jt, ts], ALU.add, ALU.add)
                            nc.vector.tensor_mul(sq[:, jt, :], r1[:, jt, :],
                                                 r1[:, jt, :])
                        psm = psS.tile([1, NT], F32, tag="small")
                        for kc in range(KC):
                            nc.tensor.matmul(psm, ones_sb, r1[:, kc, :],
                                             start=(kc == 0), stop=(kc == KC - 1))
                        psq2 = psS.tile([1, NT], F32, tag="small")
                        for kc in range(KC):
                            nc.tensor.matmul(psq2, ones_sb, sq[:, kc, :],
                                             start=(kc == 0), stop=(kc == KC - 1))
                        mean = pool.tile([1, NT], F32, tag="mean")
                        nc.vector.tensor_scalar_mul(mean, psm, 1.0 / D)
                        var = pool.tile([1, NT], F32, tag="var")
                        nc.vector.tensor_scalar_mul(var, psq2, 1.0 / D)
                        msq = pool.tile([1, NT], F32, tag="msq")
                        nc.vector.tensor_mul(msq, mean, mean)
                        nc.vector.tensor_sub(var, var, msq)
                        std = pool.tile([1, NT], F32, tag="std")
                        nc.scalar.activation(std, var, AF.Sqrt, bias=eps_sb)
                        rstd = pool.tile([1, NT], F32, tag="rstd")
                        nc.vector.reciprocal(rstd, std)
                        nmr = pool.tile([1, NT], F32, tag="nmr")
                        nc.vector.scalar_tensor_tensor(nmr, mean, -1.0, rstd,
                                                       ALU.mult, ALU.mult)
                        rsb = pool.tile([128, NT], F32, tag="rsb")
                        _bcast(nc, rsb, rstd, 128)
                        nsb = pool.tile([128, NT], F32, tag="nsb")
                        _bcast(nc, nsb, nmr, 128)
                        ln1 = pool.tile([128, KC, NT], F32R, tag="ln1")
                        for kc in range(KC):
                            tmp = pool2.tile([128, NT], F32, tag="lntmp")
                            nc.vector.tensor_mul(tmp, r1[:, kc, :], rsb)
                            nc.vector.tensor_add(tmp, tmp, nsb)
                            nc.vector.tensor_scalar(
                                ln1[:, kc, :], tmp, g1_sb[:, kc:kc + 1],
                                be1_sb[:, kc:kc + 1], ALU.mult, ALU.add)
                        nc.sync.dma_start(
                            out=d_ln1.rearrange("(c p) t -> p c t", p=128)[
                                :, :, tok0 + qb * NT:tok0 + (qb + 1) * NT],
                            in_=ln1)

            # ================= phase 2 =================
            with ExitStack() as p2:
                wpool = p2.enter_context(tc.tile_pool(name="w12", bufs=1))
                pool = p2.enter_context(tc.tile_pool(name="p2", bufs=1))
                pool2 = p2.enter_context(tc.tile_pool(name="p2b", bufs=2))
                psF = p2.enter_context(tc.tile_pool(name="psF", bufs=4, space="PSUM"))
                psT = p2.enter_context(tc.tile_pool(name="psT", bufs=2, space="PSUM"))

                w1_sb = wpool.tile([128, KC, F], F32R)
                nc.sync.dma_start(out=w1_sb,
                                  in_=d_w1T.rearrange("(c p) j -> p c j", p=128))
                w2_sb = wpool.tile([128, FC, D], F32R)
                nc.sync.dma_start(out=w2_sb,
                                  in_=d_w2T.rearrange("(c p) j -> p c j", p=128))

                for tb in range(TB):
                    ts_abs = bass.ds(tb * NT, NT)
                    ln1t = pool.tile([128, KC, NT], F32R, tag="ln1t")
                    nc.sync.dma_start(
                        out=ln1t,
                        in_=d_ln1.rearrange("(c p) t -> p c t", p=128)[:, :, ts_abs])
                    h1 = pool2.tile([128, FC, NT], F32R, tag="h1")
                    for ft in range(FC):
                        ps = psF.tile([128, NT], F32, tag="ffn")
                        for kc in range(KC):
                            nc.tensor.matmul(
                                ps, w1_sb[:, kc, bass.ts(ft, 128)], ln1t[:, kc, :],
                                start=(kc == 0), stop=(kc == KC - 1))
                        nc.scalar.activation(h1[:, ft, :], ps, gelu_func,
                                             bias=bf1_sb[:, ft:ft + 1])
                    r2 = pool.tile([128, KC, NT], F32R, tag="r2")
                    sq2 = pool.tile([128, KC, NT], F32R, tag="sq2")
                    for jt in range(JT):
                        ps = psF.tile([128, NT], F32, tag="ffn")
                        for fc in range(FC):
                            nc.tensor.matmul(
                                ps, w2_sb[:, fc, bass.ts(jt, 128)], h1[:, fc, :],
                                start=(fc == 0), stop=(fc == FC - 1))
                        g2t = pool.tile([128, NT], F32, tag="gelu2")
                        nc.scalar.activation(g2t, ps, gelu_func,
                                             bias=bf2_sb[:, jt:jt + 1])
                        nc.vector.tensor_add(r2[:, jt, :], ln1t[:, jt, :], g2t)
                        nc.vector.tensor_mul(sq2[:, jt, :], r2[:, jt, :], r2[:, jt, :])
                    psm = psT.tile([1, NT], F32, tag="small2")
                    for kc in range(KC):
                        nc.tensor.matmul(psm, ones_sb, r2[:, kc, :],
                                         start=(kc == 0), stop=(kc == KC - 1))
                    psq2 = psT.tile([1, NT], F32, tag="small2")
                    for kc in range(KC):
                        nc.tensor.matmul(psq2, ones_sb, sq2[:, kc, :],
                                         start=(kc == 0), stop=(kc == KC - 1))
                    mean = pool.tile([1, NT], F32, tag="mean2")
                    nc.vector.tensor_scalar_mul(mean, psm, 1.0 / D)
                    var = pool.tile([1, NT], F32, tag="var2")
                    nc.vector.tensor_scalar_mul(var, psq2, 1.0 / D)
                    msq = pool.tile([1, NT], F32, tag="msq2")
                    nc.vector.tensor_mul(msq, mean, mean)
                    nc.vector.tensor_sub(var, var, msq)
                    std = pool.tile([1, NT], F32, tag="std2")
                    nc.scalar.activation(std, var, AF.Sqrt, bias=eps_sb)
                    rstd = pool.tile([1, NT], F32, tag="rstd2")
                    nc.vector.reciprocal(rstd, std)
                    nmr = pool.tile([1, NT], F32, tag="nmr2")
                    nc.vector.scalar_tensor_tensor(nmr, mean, -1.0, rstd,
                                                   ALU.mult, ALU.mult)
                    rsb = pool2.tile([128, NT], F32, tag="rsb2")
                    _bcast(nc, rsb, rstd, 128)
                    nsb = pool2.tile([128, NT], F32, tag="nsb2")
                    _bcast(nc, nsb, nmr, 128)
                    gb = pool2.tile([128, NT], F32, tag="gb")
                    nc.sync.dma_start(
                        out=gb.rearrange("p (o d) -> p o d", o=1),
                        in_=d_g2d[:, ts_abs].partition_broadcast(128))
                    yt = pool.tile([128, KC, NT], F32, tag="yt")
                    for kc in range(KC):
                        tmp = pool.tile([128, NT], F32, tag="lntmp2")
                        nc.vector.tensor_mul(tmp, r2[:, kc, :], rsb)
                        nc.vector.tensor_add(tmp, tmp, nsb)
                        nc.vector.tensor_scalar(
                            tmp, tmp, g2_sb[:, kc:kc + 1], be2_sb[:, kc:kc + 1],
                            ALU.mult, ALU.add)
                        nc.vector.tensor_mul(yt[:, kc, :], tmp, gb)
                    nc.sync.dma_start(
                        out=d_yT.rearrange("(c p) t -> p c t", p=128)[:, :, ts_abs],
                        in_=yt)

    nc.compile()
    return nc


def make_in_map(inputs, e, B=4, S=1024, D=512, F=2048, E=8, NH=8):
    """Host-side input marshalling for core `e` (expert `e`)."""
    import numpy as np
    HD = D // NH
    f32 = np.float32
    x = np.ascontiguousarray(np.asarray(inputs["x"], f32).reshape(-1, D).T)
    Wqkv = np.asarray(inputs["Wqkv"][e], f32)
    bqkv = np.asarray(inputs["bqkv"][e], f32)
    WqkvT = Wqkv.T
    scale = f32(1.0 / np.sqrt(HD))
    return {
        "xT": x,
        "gwkT": np.ascontiguousarray(np.asarray(inputs["gate_Wk"], f32).T),
        "gbk": np.asarray(inputs["gate_bk"], f32),
        "eqT": np.ascontiguousarray(
            np.asarray(inputs["expert_queries"], f32).T * f32(D ** -0.5)),
        "sel": np.eye(E, dtype=f32)[e],
        "wqT": np.ascontiguousarray(WqkvT[:, :D] * scale),
        "wkT": np.ascontiguousarray(WqkvT[:, D:2 * D]),
        "wvT": np.ascontiguousarray(WqkvT[:, 2 * D:]),
        "bq": np.ascontiguousarray(bqkv[:D] * scale),
        "bk": np.ascontiguousarray(bqkv[D:2 * D]),
        "bv": np.ascontiguousarray(bqkv[2 * D:]),
        "woT": np.ascontiguousarray(np.asarray(inputs["Wo"][e], f32).T),
        "bo": np.asarray(inputs["bo"][e], f32),
        "g1": np.asarray(inputs["g1"][e], f32),
        "be1": np.asarray(inputs["be1"][e], f32),
        "w1T": np.ascontiguousarray(np.asarray(inputs["W1"][e], f32).T),
        "bf1": np.asarray(inputs["bf1"][e], f32),
        "w2T": np.ascontiguousarray(np.asarray(inputs["W2"][e], f32).T),
        "bf2": np.asarray(inputs["bf2"][e], f32),
        "g2": np.asarray(inputs["g2"][e], f32),
        "be2": np.asarray(inputs["be2"][e], f32),
    }


class SpmdRunner:
    def __init__(self, nc, n_cores=8):
        import jax
        from jax.sharding import Mesh, PartitionSpec, NamedSharding
        from jax.experimental.shard_map import shard_map
        import concourse.mybir as mybir
        from concourse import bass2jax

        bass2jax.install_neuronx_cc_hook()
        self.jax = jax
        self.nc = nc
        self.n_cores = n_cores

        partition_name = (nc.partition_id_tensor.name
                          if nc.partition_id_tensor else None)
        in_names, out_names, out_avals, zero_outs = [], [], [], []
        for alloc in nc.m.functions[0].allocations:
            if not isinstance(alloc, mybir.MemoryLocationSet):
                continue
            name = alloc.memorylocations[0].name
            if alloc.kind == "ExternalInput":
                if name != partition_name:
                    in_names.append(name)
            elif alloc.kind == "ExternalOutput":
                shape = tuple(alloc.tensor_shape)
                dtype = mybir.dt.np(alloc.dtype)
                out_names.append(name)
                out_avals.append(jax.core.ShapedArray(shape, dtype))
                zero_outs.append(np.zeros(shape, dtype))
        self.in_names, self.out_names = in_names, out_names
        self.out_avals, self.zero_outs = out_avals, zero_outs
        n_params, n_outs = len(in_names), len(out_names)
        all_in_names = list(in_names) + list(out_names)
        if partition_name is not None:
            all_in_names.append(partition_name)

        def _body(*args):
            operands = list(args)
            if partition_name is not None:
                operands.append(bass2jax.partition_id_tensor())
            outs = bass2jax._bass_exec_p.bind(
                *operands,
                out_avals=tuple(out_avals),
                in_names=tuple(all_in_names),
                out_names=tuple(out_names),
                lowering_input_output_aliases=(),
                sim_require_finite=True,
                sim_require_nnan=True,
                nc=nc,
            )
            return tuple(outs)

        devices = jax.devices()[:n_cores]
        assert len(devices) == n_cores
        self.mesh = Mesh(np.asarray(devices), ("core",))
        specs = (PartitionSpec("core"),) * (n_params + n_outs)
        out_specs = (PartitionSpec("core"),) * n_outs
        self.sharding = NamedSharding(self.mesh, PartitionSpec("core"))
        self.fn = jax.jit(
            shard_map(_body, mesh=self.mesh, in_specs=specs,
                      out_specs=out_specs, check_rep=False),
            keep_unused=True)
        self._dev_args = None

    def set_inputs(self, in_maps):
        """in_maps: list of dicts (one per core). Transfers to device once."""
        jax = self.jax
        per_core = [[np.asarray(m[name]) for name in self.in_names]
                    for m in in_maps]
        concat = [np.concatenate([per_core[c][i] for c in range(self.n_cores)],
                                 axis=0)
                  for i in range(len(self.in_names))]
        concat += [np.zeros((self.n_cores * z.shape[0], *z.shape[1:]), z.dtype)
                   for z in self.zero_outs]
        self._dev_args = [jax.device_put(a, self.sharding) for a in concat]
        return self

    def run(self):
        outs = self.fn(*self._dev_args)
        self.jax.block_until_ready(outs)
        return outs

    def results(self, outs):
        out = []
        for c in range(self.n_cores):
            d = {}
            for i, name in enumerate(self.out_names):
                d[name] = np.asarray(outs[i]).reshape(
                    self.n_cores, *self.out_avals[i].shape)[c]
            out.append(d)
        return out


_CACHE = {}


def _get_runner():
    if "r" not in _CACHE:
        nc = build()
        _CACHE["r"] = SpmdRunner(nc, 8)
    return _CACHE["r"]


def kernel(**inputs):
    B, S, D, E = 4, 1024, 512, 8
    inputs = {k: np.asarray(v) for k, v in inputs.items()}
    r = _get_runner()
    in_maps = [make_in_map(inputs, e) for e in range(E)]
    r.set_inputs(in_maps)
    outs = r.run()
    res = r.results(outs)
    yT = res[0]["yT"].astype(np.float64)
    for e in range(1, E):
        yT += res[e]["yT"].astype(np.float64)
    return np.ascontiguousarray(yT.T).reshape(B, S, D).astype(np.float32)

